# revision 6
# baseline (speedup 1.0000x reference)
"""FPN ROIAlign pooler (nn_Pooler) on 8 trn2 cores — TensorE version.

Host builds a channels-last bf16 pixel table and per-box separable bilinear
weights collapsed into dense per-chunk weight matrices W[128slots, q*49].
Device, per box: dynamic-offset strided DMAs load the footprint as q-pixel
"slots" (q in {2,4,8} -> 1-4KB descriptors) into [128, nch, q*256] tiles;
q*nch matmuls W^T @ patch accumulate pooled[49, 256] in PSUM; DVE evacuates
(cast bf16) into staged groups DMAd out. Offsets are loaded 16-at-a-time into
registers (one TENSOR_LOAD each). Boxes sorted per level and dealt
round-robin so all 8 cores run one shared program; per-rank shape covers are
maxed over cores.
"""
import numpy as np
import ml_dtypes
from contextlib import ExitStack

from concourse import bacc, bass, mybir, tile, bass_utils

nbf16 = ml_dtypes.bfloat16
BF16 = mybir.dt.bfloat16
C = 256
OUT = 7
NBIN = OUT * OUT
N_CORES = 8
LVL_HW = [(200, 304), (100, 152), (50, 76), (25, 38)]
SCALES = (0.25, 0.125, 0.0625, 0.03125)
SEG_BASE = np.zeros((4, 2), np.int64)
_off = 0
for _l in range(4):
    for _b in range(2):
        SEG_BASE[_l, _b] = _off
        _off += LVL_HW[_l][0] * LVL_HW[_l][1]
TOTAL_PX = int(_off)
PAD_PX = 16384
TABLE_PX = TOTAL_PX + PAD_PX

QS = (2, 4, 8)
VLOAD_B = 16           # offsets per batched register load
WBLK_COLS = 4096       # W streaming block width (bf16 cols)
OUT_GRP = 8            # slots per output staging group

_nc_cache = {}


def _geometry(boxes, bidx):
    """Per-box level routing + footprint + separable bilinear weights."""
    boxes32 = np.asarray(boxes, np.float32)
    b = np.asarray(bidx).astype(np.int64)
    N = boxes32.shape[0]

    x1, y1, x2, y2 = (boxes32[:, k] for k in range(4))
    area = (x2 - x1 + np.float32(1.0)) * (y2 - y1 + np.float32(1.0))
    s = np.sqrt(area)
    lv = np.floor(np.float32(4.0) + np.log2(s / np.float32(224.0)
                                            + np.float32(1e-6)))
    lvl = (np.clip(lv, 2.0, 5.0)).astype(np.int64) - 2

    scs = np.array(SCALES)[lvl]
    Wl = np.array([hw[1] for hw in LVL_HW])[lvl]
    Hl = np.array([hw[0] for hw in LVL_HW])[lvl]
    x1s = boxes32[:, 0].astype(np.float64) * scs
    y1s = boxes32[:, 1].astype(np.float64) * scs
    x2s = boxes32[:, 2].astype(np.float64) * scs
    y2s = boxes32[:, 3].astype(np.float64) * scs
    bin_w = np.maximum(x2s - x1s, 1.0) / OUT
    bin_h = np.maximum(y2s - y1s, 1.0) / OUT
    grid = (np.arange(OUT)[:, None]
            + np.array([0.25, 0.75])[None, :]).reshape(-1)
    xs = x1s[:, None] + bin_w[:, None] * grid[None, :]     # [N,14]
    ys = y1s[:, None] + bin_h[:, None] * grid[None, :]
    vx = (xs >= -1.0) & (xs <= Wl[:, None])
    vy = (ys >= -1.0) & (ys <= Hl[:, None])
    xc = np.clip(xs, 0.0, (Wl - 1)[:, None])
    yc = np.clip(ys, 0.0, (Hl - 1)[:, None])
    x0c = np.minimum(np.floor(xc).astype(np.int64), (Wl - 2)[:, None])
    y0c = np.minimum(np.floor(yc).astype(np.int64), (Hl - 2)[:, None])
    lx = xc - x0c
    ly = yc - y0c

    xmin = x0c.min(1)
    ymin = y0c.min(1)
    w_ext = x0c.max(1) + 1 - xmin + 1
    h_ext = y0c.max(1) + 1 - ymin + 1

    A_list, B_list = [], []
    for i in range(N):
        A = np.zeros((OUT, int(h_ext[i])), np.float64)
        Bm = np.zeros((OUT, int(w_ext[i])), np.float64)
        ry = y0c[i] - ymin[i]
        rx = x0c[i] - xmin[i]
        wy0 = 0.5 * vy[i] * (1.0 - ly[i])
        wy1 = 0.5 * vy[i] * ly[i]
        wx0 = 0.5 * vx[i] * (1.0 - lx[i])
        wx1 = 0.5 * vx[i] * lx[i]
        for sy in range(2 * OUT):
            by = sy // 2
            A[by, ry[sy]] += wy0[sy]
            A[by, ry[sy] + 1] += wy1[sy]
        for sx in range(2 * OUT):
            bx = sx // 2
            Bm[bx, rx[sx]] += wx0[sx]
            Bm[bx, rx[sx] + 1] += wx1[sx]
        A_list.append(A.astype(np.float32))
        B_list.append(Bm.astype(np.float32))

    return dict(lvl=lvl, b=b, xmin=xmin, ymin=ymin, w_ext=w_ext,
                h_ext=h_ext, A=A_list, B=B_list)


def _best_q(w, h):
    best = None
    for q in QS:
        wq = -(-w // q)
        k = 128 // wq
        if k == 0:
            continue
        nch = -(-h // k)
        cost = 1.1 * nch * 128 * q + 1.43 * h * wq * q + 8.0 * h * wq
        if best is None or cost < best[0]:
            best = (cost, q)
    return best[1]


def _plan(geo):
    """Sort + deal boxes, compute per-rank shape covers."""
    lvl = geo["lvl"]
    w_ext = geo["w_ext"]
    h_ext = geo["h_ext"]
    N = len(w_ext)
    qsel = np.array([_best_q(int(w_ext[i]), int(h_ext[i]))
                     for i in range(N)])
    wqsel = np.array([-(-int(w_ext[i]) // int(qsel[i])) for i in range(N)])

    slots = []        # dict(lvl, q, wq, k, nch, k_last, Wl)
    slot_boxes = []   # [box_id or -1 per core]
    for l in range(4):
        ids = np.nonzero(lvl == l)[0]
        order = ids[np.lexsort((-h_ext[ids], -wqsel[ids], -qsel[ids]))]
        n_ranks = -(-len(order) // N_CORES)
        for j in range(n_ranks):
            grp = order[j * N_CORES:(j + 1) * N_CORES]
            cores_boxes = [-1] * N_CORES
            for c, bid in enumerate(grp):
                cores_boxes[c] = int(bid)
            qj = int(qsel[grp].max())
            wqj = int(max(-(-int(w_ext[g]) // qj) for g in grp))
            kj = 128 // wqj
            nchj = int(max(-(-int(h_ext[g]) // kj) for g in grp))
            klj = max(1, int(max(int(h_ext[g]) - (nchj - 1) * kj
                                 for g in grp)))
            slots.append(dict(lvl=l, q=qj, wq=wqj, k=kj, nch=nchj,
                              k_last=klj, Wl=LVL_HW[l][1]))
            slot_boxes.append(cores_boxes)
    return slots, slot_boxes


def _host_prep(f0, f1, f2, f3, boxes, bidx):
    geo = _geometry(boxes, bidx)
    slots, slot_boxes = _plan(geo)
    n_slots = len(slots)

    # channels-last bf16 table
    segs = []
    for f in (f0, f1, f2, f3):
        fa = np.asarray(f, np.float32)
        for bb in range(2):
            segs.append(np.transpose(fa[bb], (1, 2, 0)).reshape(-1, C))
    segs.append(np.zeros((PAD_PX, C), np.float32))
    table = np.ascontiguousarray(
        np.concatenate(segs, 0)).astype(nbf16).reshape(-1)

    # chunk bookkeeping: engine = slot parity; meta cols ordered
    # [all engine-0 chunks in emission order | all engine-1 chunks]
    eng_nch = [0, 0]
    wcols = 0
    for si, s in enumerate(slots):
        eng_nch[si % 2] += s["nch"]
        wcols += s["nch"] * s["q"] * NBIN
    tot_chunks = eng_nch[0] + eng_nch[1]

    metas = [np.zeros((1, tot_chunks), np.int32) for _ in range(N_CORES)]
    whosts = [np.zeros((128, wcols), np.float32) for _ in range(N_CORES)]

    ecur = [0, eng_nch[0]]
    colbase = 0
    for si, s in enumerate(slots):
        q, wq, k, nch, klast, Wl = (s["q"], s["wq"], s["k"], s["nch"],
                                    s["k_last"], s["Wl"])
        e = si % 2
        mbase = ecur[e]
        ecur[e] += nch
        rows_tot = (nch - 1) * k + klast
        for core in range(N_CORES):
            bid = slot_boxes[si][core]
            if bid < 0:
                continue
            l, bimg = geo["lvl"][bid], geo["b"][bid]
            base_px = (SEG_BASE[l, bimg] + geo["ymin"][bid] * Wl
                       + geo["xmin"][bid])
            for cch in range(nch):
                metas[core][0, mbase + cch] = (base_px + cch * k * Wl) * C
            h = int(geo["h_ext"][bid])
            w = int(geo["w_ext"][bid])
            Ap = np.zeros((OUT, rows_tot), np.float32)
            Ap[:, :h] = geo["A"][bid][:, :min(h, rows_tot)]
            Bp = np.zeros((OUT, wq * q), np.float32)
            Bp[:, :w] = geo["B"][bid]
            T = np.einsum('ar,bd->rdab', Ap, Bp).reshape(rows_tot, wq, q,
                                                         NBIN)
            wh = whosts[core]
            for cch in range(nch):
                rows_c = k if cch < nch - 1 else klast
                arr = T[cch * k:cch * k + rows_c]      # [rows_c, wq, q, 49]
                used = rows_c * wq
                cb = colbase + cch * q * NBIN
                wh[:used, cb:cb + q * NBIN] = arr.reshape(used, q * NBIN)
        colbase += nch * q * NBIN
    assert colbase == wcols

    whosts = [w.astype(nbf16) for w in whosts]
    key = tuple((s["lvl"], s["q"], s["wq"], s["nch"], s["k_last"])
                for s in slots)
    return (table, metas, whosts, slots, slot_boxes, tot_chunks, eng_nch,
            n_slots, key)


def _build_nc(slots, eng_nch):
    tot_chunks = eng_nch[0] + eng_nch[1]
    n_slots = len(slots)
    wcols = sum(s["nch"] * s["q"] * NBIN for s in slots)
    nc = bacc.Bacc("TRN2", target_bir_lowering=False, debug=False,
                   num_devices=N_CORES)
    table_d = nc.dram_tensor("table", [TABLE_PX * C], BF16,
                             kind="ExternalInput")
    meta_d = nc.dram_tensor("meta", [1, tot_chunks], mybir.dt.int32,
                            kind="ExternalInput")
    w_d = nc.dram_tensor("wts", [128, wcols], BF16, kind="ExternalInput")
    out_d = nc.dram_tensor("out", [NBIN, n_slots, C], BF16,
                           kind="ExternalOutput")

    with tile.TileContext(nc) as tc, ExitStack() as ctx:
        sbm = ctx.enter_context(tc.tile_pool(name="sbm", bufs=1))
        sbw = ctx.enter_context(tc.tile_pool(name="sbw", bufs=2))
        sbp = ctx.enter_context(tc.tile_pool(name="sbp", bufs=4))
        sbo = ctx.enter_context(tc.tile_pool(name="sbo", bufs=2))
        psp = ctx.enter_context(tc.psum_pool(name="psp", bufs=8))

        meta_t = sbm.tile([1, tot_chunks], mybir.dt.int32)
        nc.sync.dma_start(out=meta_t[:], in_=meta_d.ap())

        engs = [nc.sync, nc.scalar]
        # batched offset loads: per engine, a rolling window of ScalarValues
        ecur = [0, eng_nch[0]]
        eoff_vals = [{}, {}]

        def get_off(e, mi):
            vals = eoff_vals[e]
            if mi not in vals:
                b0 = mi - (mi - (0 if e == 0 else eng_nch[0])) % VLOAD_B
                n = min(VLOAD_B,
                        (eng_nch[0] if e == 0 else tot_chunks) - b0)
                regs = [engs[e].alloc_register(f"off{e}_{b0}_{i}")
                        for i in range(n)]
                engs[e].reg_load(regs, meta_t[0:1, b0:b0 + n])
                for i, r in enumerate(regs):
                    vals[b0 + i] = engs[e].snap(r, donate=True)
            return vals.pop(mi)

        # W streaming blocks at chunk granularity
        wblocks = []        # (colstart, colend)
        chunk_wcol = []     # per global chunk-col ranges are tracked inline
        wtiles = {}

        # precompute chunk -> (block, col) assignment
        col = 0
        chunk_cols = []     # per (si, cch): col
        for s in slots:
            for cch in range(s["nch"]):
                chunk_cols.append((col, s["q"] * NBIN))
                col += s["q"] * NBIN
        blocks = []
        b0 = 0
        for gci, (c0, cw) in enumerate(chunk_cols):
            if c0 + cw - b0 > WBLK_COLS:
                blocks.append((b0, c0))
                b0 = c0
        blocks.append((b0, wcols))
        chunk_blk = {}
        for gci, (c0, cw) in enumerate(chunk_cols):
            for bi, (lo, hi) in enumerate(blocks):
                if lo <= c0 < hi:
                    chunk_blk[gci] = bi
                    break

        def get_wblk(bi):
            if bi not in wtiles:
                lo, hi = blocks[bi]
                t = sbw.tile([128, hi - lo], BF16)
                nc.sync.dma_start(out=t[:], in_=w_d.ap()[:, lo:hi])
                wtiles[bi] = (t, lo)
            return wtiles[bi]

        stage = None
        gsz = 0
        gci = 0
        for si, s in enumerate(slots):
            q, wq, k, nch, klast, Wl = (s["q"], s["wq"], s["k"], s["nch"],
                                        s["k_last"], s["Wl"])
            e = si % 2
            eng = engs[e]
            qC = q * C
            patch = sbp.tile([128, nch, qC], BF16)
            used_min = klast * wq
            for g in range(used_min // 32, 4):
                nc.vector.memset(patch[g * 32:(g + 1) * 32, :, :], 0.0)
            for cch in range(nch):
                rows_c = k if cch < nch - 1 else klast
                off = get_off(e, ecur[e])
                ecur[e] += 1
                src = bass.AP(tensor=table_d, offset=off,
                              ap=[[Wl * C, rows_c], [qC, wq], [1, qC]])
                eng.dma_start(out=patch[:rows_c * wq, cch, :], in_=src)
            ps = psp.tile([NBIN, C], mybir.dt.float32)
            for cch in range(nch):
                bi = chunk_blk[gci]
                wt, wlo = get_wblk(bi)
                c0 = chunk_cols[gci][0] - wlo
                for sub in range(q):
                    nc.tensor.matmul(
                        out=ps[:],
                        lhsT=wt[:, c0 + sub * NBIN:c0 + (sub + 1) * NBIN],
                        rhs=patch[:, cch, sub * C:(sub + 1) * C],
                        start=(cch == 0 and sub == 0),
                        stop=(cch == nch - 1 and sub == q - 1))
                gci += 1
            gi = si % OUT_GRP
            if gi == 0:
                gsz = min(OUT_GRP, n_slots - si)
                stage = sbo.tile([NBIN, gsz, C], BF16)
            nc.vector.tensor_scalar_mul(stage[:, gi, :], ps[:], 1.0)
            if gi == gsz - 1:
                g0 = si - gi
                nc.scalar.dma_start(
                    out=out_d.ap()[:, g0:g0 + gsz, :], in_=stage[:])
    nc.compile()
    return nc


LAST_RESULT = None


def kernel(f0, f1, f2, f3, boxes, box_batch_idx):
    global LAST_RESULT
    (table, metas, whosts, slots, slot_boxes, tot_chunks, eng_nch, n_slots,
     key) = _host_prep(f0, f1, f2, f3, boxes, box_batch_idx)
    if key not in _nc_cache:
        _nc_cache[key] = _build_nc(slots, eng_nch)
    nc = _nc_cache[key]
    in_maps = [{"table": table, "meta": metas[i], "wts": whosts[i]}
               for i in range(N_CORES)]
    res = bass_utils.run_bass_kernel_spmd(nc, in_maps,
                                          core_ids=list(range(N_CORES)))
    LAST_RESULT = res

    outfull = np.zeros((1024, NBIN, C), np.float32)
    for core in range(N_CORES):
        r = np.asarray(res.results[core]["out"]).astype(np.float32)
        r = r.reshape(NBIN, n_slots, C)
        for si in range(n_slots):
            bid = slot_boxes[si][core]
            if bid >= 0:
                outfull[bid] = r[:, si, :]
    return np.ascontiguousarray(
        outfull.transpose(0, 2, 1).reshape(1024, C, OUT, OUT))


# revision 19
# speedup vs baseline: 1.5044x; 1.5044x over previous
"""FPN ROIAlign pooler (nn_Pooler) on 8 trn2 cores — TensorE version.

Host builds a channels-last bf16 pixel table and per-box separable bilinear
weights collapsed into dense per-chunk weight matrices W[128slots, q*49].
Device, per box: dynamic-offset strided DMAs load the footprint as q-pixel
"slots" (q in {2,4,8} -> 1-4KB descriptors) into [128, nch, q*256] tiles;
q*nch matmuls W^T @ patch accumulate pooled[49, 256] in PSUM; DVE evacuates
(cast bf16) into staged groups DMAd out. Offsets are loaded 16-at-a-time into
registers (one TENSOR_LOAD each). Boxes sorted per level and dealt
round-robin so all 8 cores run one shared program; per-rank shape covers are
maxed over cores.
"""
import numpy as np
import ml_dtypes
from contextlib import ExitStack

from concourse import bacc, bass, mybir, tile, bass_utils

nbf16 = ml_dtypes.bfloat16
BF16 = mybir.dt.bfloat16
C = 256
OUT = 7
NBIN = OUT * OUT
N_CORES = 8
LVL_HW = [(200, 304), (100, 152), (50, 76), (25, 38)]
SCALES = (0.25, 0.125, 0.0625, 0.03125)
SEG_BASE = np.zeros((4, 2), np.int64)
_off = 0
for _l in range(4):
    for _b in range(2):
        SEG_BASE[_l, _b] = _off
        _off += LVL_HW[_l][0] * LVL_HW[_l][1]
TOTAL_PX = int(_off)
PAD_PX = 16384
TABLE_PX = TOTAL_PX + PAD_PX

QS = (2, 4, 8)
VLOAD_B = 16           # offsets per batched register load
WBLK_COLS = 12288      # W streaming block width (bf16 cols)
OUT_GRP = 8            # slots per output staging group
PFREE = 4096           # constant patch-tile free dim (elems, bf16)
PATCH_BUFS = 4

_nc_cache = {}


def _geometry(boxes, bidx):
    """Per-box level routing + footprint + separable bilinear weights."""
    boxes32 = np.asarray(boxes, np.float32)
    b = np.asarray(bidx).astype(np.int64)
    N = boxes32.shape[0]

    x1, y1, x2, y2 = (boxes32[:, k] for k in range(4))
    area = (x2 - x1 + np.float32(1.0)) * (y2 - y1 + np.float32(1.0))
    s = np.sqrt(area)
    lv = np.floor(np.float32(4.0) + np.log2(s / np.float32(224.0)
                                            + np.float32(1e-6)))
    lvl = (np.clip(lv, 2.0, 5.0)).astype(np.int64) - 2

    scs = np.array(SCALES)[lvl]
    Wl = np.array([hw[1] for hw in LVL_HW])[lvl]
    Hl = np.array([hw[0] for hw in LVL_HW])[lvl]
    x1s = boxes32[:, 0].astype(np.float64) * scs
    y1s = boxes32[:, 1].astype(np.float64) * scs
    x2s = boxes32[:, 2].astype(np.float64) * scs
    y2s = boxes32[:, 3].astype(np.float64) * scs
    bin_w = np.maximum(x2s - x1s, 1.0) / OUT
    bin_h = np.maximum(y2s - y1s, 1.0) / OUT
    grid = (np.arange(OUT)[:, None]
            + np.array([0.25, 0.75])[None, :]).reshape(-1)
    xs = x1s[:, None] + bin_w[:, None] * grid[None, :]     # [N,14]
    ys = y1s[:, None] + bin_h[:, None] * grid[None, :]
    vx = (xs >= -1.0) & (xs <= Wl[:, None])
    vy = (ys >= -1.0) & (ys <= Hl[:, None])
    xc = np.clip(xs, 0.0, (Wl - 1)[:, None])
    yc = np.clip(ys, 0.0, (Hl - 1)[:, None])
    x0c = np.minimum(np.floor(xc).astype(np.int64), (Wl - 2)[:, None])
    y0c = np.minimum(np.floor(yc).astype(np.int64), (Hl - 2)[:, None])
    lx = xc - x0c
    ly = yc - y0c

    xmin = x0c.min(1)
    ymin = y0c.min(1)
    w_ext = x0c.max(1) + 1 - xmin + 1
    h_ext = y0c.max(1) + 1 - ymin + 1

    A_list, B_list = [], []
    for i in range(N):
        A = np.zeros((OUT, int(h_ext[i])), np.float64)
        Bm = np.zeros((OUT, int(w_ext[i])), np.float64)
        ry = y0c[i] - ymin[i]
        rx = x0c[i] - xmin[i]
        wy0 = 0.5 * vy[i] * (1.0 - ly[i])
        wy1 = 0.5 * vy[i] * ly[i]
        wx0 = 0.5 * vx[i] * (1.0 - lx[i])
        wx1 = 0.5 * vx[i] * lx[i]
        for sy in range(2 * OUT):
            by = sy // 2
            A[by, ry[sy]] += wy0[sy]
            A[by, ry[sy] + 1] += wy1[sy]
        for sx in range(2 * OUT):
            bx = sx // 2
            Bm[bx, rx[sx]] += wx0[sx]
            Bm[bx, rx[sx] + 1] += wx1[sx]
        A_list.append(A.astype(np.float32))
        B_list.append(Bm.astype(np.float32))

    return dict(lvl=lvl, b=b, xmin=xmin, ymin=ymin, w_ext=w_ext,
                h_ext=h_ext, A=A_list, B=B_list)


def _shape_cost(w, h, q, wq):
    """ns-scale cost of covering a (w x h) footprint with q-px slots,
    wq slots per row: SDMA bytes (reads + W stream) + PE + descriptors."""
    k = 128 // wq
    if k == 0:
        return None
    nch = -(-h // k)
    if nch * q * C > PFREE:
        return None
    read_b = h * wq * q * 512
    w_b = nch * 128 * q * 98
    pe = nch * q * 305 / 2.4
    return (read_b + w_b) / 358.0 + pe + 8.0 * h * wq


def _best_qwq(w, h):
    best = None
    for q in QS:
        wq0 = -(-w // q)
        if wq0 > 128:
            continue
        for wq in range(wq0, 129):
            c = _shape_cost(w, h, q, wq)
            if c is not None and (best is None or c < best[0]):
                best = (c, q, wq)
    return best[1], best[2]


def _plan(geo):
    """Sort + deal boxes, compute per-rank shape covers."""
    lvl = geo["lvl"]
    w_ext = geo["w_ext"]
    h_ext = geo["h_ext"]
    N = len(w_ext)
    qw = [_best_qwq(int(w_ext[i]), int(h_ext[i])) for i in range(N)]
    qsel = np.array([t[0] for t in qw])
    wqsel = np.array([t[1] for t in qw])

    slots = []        # dict(lvl, q, wq, k, nch, k_last, Wl)
    slot_boxes = []   # [box_id or -1 per core]

    def cover_of(grp):
        w_max = int(max(w_ext[g] for g in grp))
        h_max = int(max(h_ext[g] for g in grp))
        best = None
        for qq in QS:
            for wq in range(-(-w_max // qq), 129):
                c = _shape_cost(w_max, h_max, qq, wq)
                if c is not None and (best is None or c < best[0]):
                    best = (c, qq, wq)
        return best

    def emit_group(l, grp):
        best = cover_of(grp)
        if best is None:
            # no single feasible shape covers this mix; split by width
            srt = sorted(grp, key=lambda g: -w_ext[g])
            half = max(1, len(srt) // 2)
            emit_group(l, srt[:half])
            emit_group(l, srt[half:])
            return
        _, qj, wqj = best
        kj = 128 // wqj
        h_max = int(max(h_ext[g] for g in grp))
        nchj = -(-h_max // kj)
        klj = max(1, h_max - (nchj - 1) * kj)
        cores_boxes = [-1] * N_CORES
        for c, bid in enumerate(grp):
            cores_boxes[c] = int(bid)
        slots.append(dict(lvl=l, q=qj, wq=wqj, k=kj, nch=nchj,
                          k_last=klj, Wl=LVL_HW[l][1]))
        slot_boxes.append(cores_boxes)

    widths = wqsel * qsel
    for l in range(4):
        ids = np.nonzero(lvl == l)[0]
        order = ids[np.lexsort((-h_ext[ids], -widths[ids]))]
        n_ranks = -(-len(order) // N_CORES)
        for j in range(n_ranks):
            emit_group(l, list(order[j * N_CORES:(j + 1) * N_CORES]))
    return slots, slot_boxes


def _host_prep(f0, f1, f2, f3, boxes, bidx):
    geo = _geometry(boxes, bidx)
    slots, slot_boxes = _plan(geo)
    n_slots = len(slots)

    # channels-last bf16 table
    segs = []
    for f in (f0, f1, f2, f3):
        fa = np.asarray(f, np.float32)
        for bb in range(2):
            segs.append(np.transpose(fa[bb], (1, 2, 0)).reshape(-1, C))
    segs.append(np.zeros((PAD_PX, C), np.float32))
    table = np.ascontiguousarray(
        np.concatenate(segs, 0)).astype(nbf16).reshape(-1)

    # chunk bookkeeping: engine = slot parity; meta cols ordered
    # [all engine-0 chunks in emission order | all engine-1 chunks]
    eng_nch = [0, 0]
    wcols = 0
    for si, s in enumerate(slots):
        eng_nch[si % 2] += s["nch"]
        wcols += s["nch"] * s["q"] * NBIN
    tot_chunks = eng_nch[0] + eng_nch[1]

    metas = [np.zeros((1, tot_chunks), np.int32) for _ in range(N_CORES)]
    whosts = [np.zeros((128, wcols), np.float32) for _ in range(N_CORES)]

    ecur = [0, eng_nch[0]]
    colbase = 0
    for si, s in enumerate(slots):
        q, wq, k, nch, klast, Wl = (s["q"], s["wq"], s["k"], s["nch"],
                                    s["k_last"], s["Wl"])
        e = si % 2
        mbase = ecur[e]
        ecur[e] += nch
        rows_tot = (nch - 1) * k + klast
        for core in range(N_CORES):
            bid = slot_boxes[si][core]
            if bid < 0:
                continue
            l, bimg = geo["lvl"][bid], geo["b"][bid]
            base_px = (SEG_BASE[l, bimg] + geo["ymin"][bid] * Wl
                       + geo["xmin"][bid])
            for cch in range(nch):
                metas[core][0, mbase + cch] = (base_px + cch * k * Wl) * C
            h = int(geo["h_ext"][bid])
            w = int(geo["w_ext"][bid])
            Ap = np.zeros((OUT, rows_tot), np.float32)
            Ap[:, :h] = geo["A"][bid][:, :min(h, rows_tot)]
            Bp = np.zeros((OUT, wq * q), np.float32)
            Bp[:, :w] = geo["B"][bid]
            T = np.einsum('ar,bd->rdab', Ap, Bp).reshape(rows_tot, wq, q,
                                                         NBIN)
            wh = whosts[core]
            for cch in range(nch):
                rows_c = k if cch < nch - 1 else klast
                arr = T[cch * k:cch * k + rows_c]      # [rows_c, wq, q, 49]
                used = rows_c * wq
                cb = colbase + cch * q * NBIN
                wh[:used, cb:cb + q * NBIN] = arr.reshape(used, q * NBIN)
        colbase += nch * q * NBIN
    assert colbase == wcols

    whosts = [w.astype(nbf16) for w in whosts]
    key = tuple((s["lvl"], s["q"], s["wq"], s["nch"], s["k_last"])
                for s in slots)
    return (table, metas, whosts, slots, slot_boxes, tot_chunks, eng_nch,
            n_slots, key)


def _build_nc(slots, eng_nch):
    tot_chunks = eng_nch[0] + eng_nch[1]
    n_slots = len(slots)
    wcols = sum(s["nch"] * s["q"] * NBIN for s in slots)
    nc = bacc.Bacc("TRN2", target_bir_lowering=False, debug=False,
                   num_devices=N_CORES)
    table_d = nc.dram_tensor("table", [TABLE_PX * C], BF16,
                             kind="ExternalInput")
    meta_d = nc.dram_tensor("meta", [1, tot_chunks], mybir.dt.int32,
                            kind="ExternalInput")
    w_d = nc.dram_tensor("wts", [128, wcols], BF16, kind="ExternalInput")
    out_d = nc.dram_tensor("out", [NBIN, n_slots, C], BF16,
                           kind="ExternalOutput")

    with tile.TileContext(nc) as tc, ExitStack() as ctx:
        sbm = ctx.enter_context(tc.tile_pool(name="sbm", bufs=1))
        sbw = ctx.enter_context(tc.tile_pool(name="sbw", bufs=2))
        sbp = ctx.enter_context(tc.tile_pool(name="sbp", bufs=PATCH_BUFS))
        sbo = ctx.enter_context(tc.tile_pool(name="sbo", bufs=2))
        psp = ctx.enter_context(tc.psum_pool(name="psp", bufs=8))

        meta_t = sbm.tile([1, tot_chunks], mybir.dt.int32)
        nc.sync.dma_start(out=meta_t[:], in_=meta_d.ap())

        engs = [nc.sync, nc.scalar]
        # batched offset loads: per engine, a rolling window of ScalarValues
        ecur = [0, eng_nch[0]]
        eoff_vals = [{}, {}]

        def get_off(e, mi):
            vals = eoff_vals[e]
            if mi not in vals:
                b0 = mi - (mi - (0 if e == 0 else eng_nch[0])) % VLOAD_B
                n = min(VLOAD_B,
                        (eng_nch[0] if e == 0 else tot_chunks) - b0)
                regs = [engs[e].alloc_register(f"off{e}_{b0}_{i}")
                        for i in range(n)]
                engs[e].reg_load(regs, meta_t[0:1, b0:b0 + n])
                for i, r in enumerate(regs):
                    vals[b0 + i] = engs[e].snap(r, donate=True)
            return vals.pop(mi)

        # W streaming blocks at chunk granularity
        wblocks = []        # (colstart, colend)
        chunk_wcol = []     # per global chunk-col ranges are tracked inline
        wtiles = {}

        # precompute chunk -> (block, col) assignment
        col = 0
        chunk_cols = []     # per (si, cch): col
        for s in slots:
            for cch in range(s["nch"]):
                chunk_cols.append((col, s["q"] * NBIN))
                col += s["q"] * NBIN
        blocks = []
        b0 = 0
        for gci, (c0, cw) in enumerate(chunk_cols):
            if c0 + cw - b0 > WBLK_COLS:
                blocks.append((b0, c0))
                b0 = c0
        blocks.append((b0, wcols))
        chunk_blk = {}
        for gci, (c0, cw) in enumerate(chunk_cols):
            for bi, (lo, hi) in enumerate(blocks):
                if lo <= c0 < hi:
                    chunk_blk[gci] = bi
                    break

        def get_wblk(bi):
            if bi not in wtiles:
                lo, hi = blocks[bi]
                t = sbw.tile([128, hi - lo], BF16)
                nc.sync.dma_start(out=t[:], in_=w_d.ap()[:, lo:hi])
                wtiles[bi] = (t, lo)
            return wtiles[bi]

        stage = None
        gsz = 0
        gci = 0
        for si, s in enumerate(slots):
            q, wq, k, nch, klast, Wl = (s["q"], s["wq"], s["k"], s["nch"],
                                        s["k_last"], s["Wl"])
            e = si % 2
            eng = engs[e]
            qC = q * C
            patch = sbp.tile([128, PFREE], BF16)
            for cch in range(nch):
                rows_c = k if cch < nch - 1 else klast
                off = get_off(e, ecur[e])
                ecur[e] += 1
                src = bass.AP(tensor=table_d, offset=off,
                              ap=[[Wl * C, rows_c], [qC, wq], [1, qC]])
                eng.dma_start(out=patch[:rows_c * wq,
                                        cch * qC:(cch + 1) * qC], in_=src)
            ps = psp.tile([NBIN, C], mybir.dt.float32)
            for cch in range(nch):
                rows_c = k if cch < nch - 1 else klast
                kc = rows_c * wq
                bi = chunk_blk[gci]
                wt, wlo = get_wblk(bi)
                c0 = chunk_cols[gci][0] - wlo
                for sub in range(q):
                    nc.tensor.matmul(
                        out=ps[:],
                        lhsT=wt[:kc, c0 + sub * NBIN:c0 + (sub + 1) * NBIN],
                        rhs=patch[:kc, cch * qC + sub * C:
                                  cch * qC + (sub + 1) * C],
                        start=(cch == 0 and sub == 0),
                        stop=(cch == nch - 1 and sub == q - 1))
                gci += 1
            gi = si % OUT_GRP
            if gi == 0:
                gsz = min(OUT_GRP, n_slots - si)
                stage = sbo.tile([NBIN, gsz, C], BF16)
            nc.vector.tensor_scalar_mul(stage[:, gi, :], ps[:], 1.0)
            if gi == gsz - 1:
                g0 = si - gi
                nc.scalar.dma_start(
                    out=out_d.ap()[:, g0:g0 + gsz, :], in_=stage[:])
    nc.compile()
    return nc


LAST_RESULT = None


def kernel(f0, f1, f2, f3, boxes, box_batch_idx):
    global LAST_RESULT
    (table, metas, whosts, slots, slot_boxes, tot_chunks, eng_nch, n_slots,
     key) = _host_prep(f0, f1, f2, f3, boxes, box_batch_idx)
    if key not in _nc_cache:
        _nc_cache[key] = _build_nc(slots, eng_nch)
    nc = _nc_cache[key]
    in_maps = [{"table": table, "meta": metas[i], "wts": whosts[i]}
               for i in range(N_CORES)]
    res = bass_utils.run_bass_kernel_spmd(nc, in_maps,
                                          core_ids=list(range(N_CORES)))
    LAST_RESULT = res

    outfull = np.zeros((1024, NBIN, C), np.float32)
    for core in range(N_CORES):
        r = np.asarray(res.results[core]["out"]).astype(np.float32)
        r = r.reshape(NBIN, n_slots, C)
        for si in range(n_slots):
            bid = slot_boxes[si][core]
            if bid >= 0:
                outfull[bid] = r[:, si, :]
    return np.ascontiguousarray(
        outfull.transpose(0, 2, 1).reshape(1024, C, OUT, OUT))


# revision 20
# speedup vs baseline: 1.6116x; 1.0713x over previous
"""FPN ROIAlign pooler (nn_Pooler) on 8 trn2 cores — TensorE version.

Host builds a channels-last bf16 pixel table and per-box separable bilinear
weights collapsed into dense per-chunk weight matrices W[128slots, q*49].
Device, per box: dynamic-offset strided DMAs load the footprint as q-pixel
"slots" (q in {2,4,8} -> 1-4KB descriptors) into [128, nch, q*256] tiles;
q*nch matmuls W^T @ patch accumulate pooled[49, 256] in PSUM; DVE evacuates
(cast bf16) into staged groups DMAd out. Offsets are loaded 16-at-a-time into
registers (one TENSOR_LOAD each). Boxes sorted per level and dealt
round-robin so all 8 cores run one shared program; per-rank shape covers are
maxed over cores.
"""
import numpy as np
import ml_dtypes
from contextlib import ExitStack

from concourse import bacc, bass, mybir, tile, bass_utils

nbf16 = ml_dtypes.bfloat16
BF16 = mybir.dt.bfloat16
C = 256
OUT = 7
NBIN = OUT * OUT
N_CORES = 8
LVL_HW = [(200, 304), (100, 152), (50, 76), (25, 38)]
SCALES = (0.25, 0.125, 0.0625, 0.03125)
SEG_BASE = np.zeros((4, 2), np.int64)
_off = 0
for _l in range(4):
    for _b in range(2):
        SEG_BASE[_l, _b] = _off
        _off += LVL_HW[_l][0] * LVL_HW[_l][1]
TOTAL_PX = int(_off)
PAD_PX = 16384
TABLE_PX = TOTAL_PX + PAD_PX

QS = (2, 4, 8)
VLOAD_B = 16           # offsets per batched register load
WBLK_COLS = 8192       # W streaming block width (bf16 cols)
OUT_GRP = 8            # slots per output staging group
PFREE = 4096           # constant patch-tile free dim (elems, bf16)
PATCH_BUFS = 6

_nc_cache = {}


def _geometry(boxes, bidx):
    """Per-box level routing + footprint + separable bilinear weights."""
    boxes32 = np.asarray(boxes, np.float32)
    b = np.asarray(bidx).astype(np.int64)
    N = boxes32.shape[0]

    x1, y1, x2, y2 = (boxes32[:, k] for k in range(4))
    area = (x2 - x1 + np.float32(1.0)) * (y2 - y1 + np.float32(1.0))
    s = np.sqrt(area)
    lv = np.floor(np.float32(4.0) + np.log2(s / np.float32(224.0)
                                            + np.float32(1e-6)))
    lvl = (np.clip(lv, 2.0, 5.0)).astype(np.int64) - 2

    scs = np.array(SCALES)[lvl]
    Wl = np.array([hw[1] for hw in LVL_HW])[lvl]
    Hl = np.array([hw[0] for hw in LVL_HW])[lvl]
    x1s = boxes32[:, 0].astype(np.float64) * scs
    y1s = boxes32[:, 1].astype(np.float64) * scs
    x2s = boxes32[:, 2].astype(np.float64) * scs
    y2s = boxes32[:, 3].astype(np.float64) * scs
    bin_w = np.maximum(x2s - x1s, 1.0) / OUT
    bin_h = np.maximum(y2s - y1s, 1.0) / OUT
    grid = (np.arange(OUT)[:, None]
            + np.array([0.25, 0.75])[None, :]).reshape(-1)
    xs = x1s[:, None] + bin_w[:, None] * grid[None, :]     # [N,14]
    ys = y1s[:, None] + bin_h[:, None] * grid[None, :]
    vx = (xs >= -1.0) & (xs <= Wl[:, None])
    vy = (ys >= -1.0) & (ys <= Hl[:, None])
    xc = np.clip(xs, 0.0, (Wl - 1)[:, None])
    yc = np.clip(ys, 0.0, (Hl - 1)[:, None])
    x0c = np.minimum(np.floor(xc).astype(np.int64), (Wl - 2)[:, None])
    y0c = np.minimum(np.floor(yc).astype(np.int64), (Hl - 2)[:, None])
    lx = xc - x0c
    ly = yc - y0c

    xmin = x0c.min(1)
    ymin = y0c.min(1)
    w_ext = x0c.max(1) + 1 - xmin + 1
    h_ext = y0c.max(1) + 1 - ymin + 1

    A_list, B_list = [], []
    for i in range(N):
        A = np.zeros((OUT, int(h_ext[i])), np.float64)
        Bm = np.zeros((OUT, int(w_ext[i])), np.float64)
        ry = y0c[i] - ymin[i]
        rx = x0c[i] - xmin[i]
        wy0 = 0.5 * vy[i] * (1.0 - ly[i])
        wy1 = 0.5 * vy[i] * ly[i]
        wx0 = 0.5 * vx[i] * (1.0 - lx[i])
        wx1 = 0.5 * vx[i] * lx[i]
        for sy in range(2 * OUT):
            by = sy // 2
            A[by, ry[sy]] += wy0[sy]
            A[by, ry[sy] + 1] += wy1[sy]
        for sx in range(2 * OUT):
            bx = sx // 2
            Bm[bx, rx[sx]] += wx0[sx]
            Bm[bx, rx[sx] + 1] += wx1[sx]
        A_list.append(A.astype(np.float32))
        B_list.append(Bm.astype(np.float32))

    return dict(lvl=lvl, b=b, xmin=xmin, ymin=ymin, w_ext=w_ext,
                h_ext=h_ext, A=A_list, B=B_list)


def _shape_cost(w, h, q, wq):
    """ns-scale cost of covering a (w x h) footprint with q-px slots,
    wq slots per row: SDMA bytes (reads + W stream) + PE + descriptors."""
    k = 128 // wq
    if k == 0:
        return None
    nch = -(-h // k)
    if nch * q * C > PFREE:
        return None
    read_b = h * wq * q * 512
    w_b = nch * 128 * q * 98
    pe = nch * q * 305 / 2.4
    return (read_b + w_b) / 358.0 + pe + 8.0 * h * wq


def _best_qwq(w, h):
    best = None
    for q in QS:
        wq0 = -(-w // q)
        if wq0 > 128:
            continue
        for wq in range(wq0, 129):
            c = _shape_cost(w, h, q, wq)
            if c is not None and (best is None or c < best[0]):
                best = (c, q, wq)
    return best[1], best[2]


def _plan(geo):
    """Sort + deal boxes, compute per-rank shape covers."""
    lvl = geo["lvl"]
    w_ext = geo["w_ext"]
    h_ext = geo["h_ext"]
    N = len(w_ext)
    qw = [_best_qwq(int(w_ext[i]), int(h_ext[i])) for i in range(N)]
    qsel = np.array([t[0] for t in qw])
    wqsel = np.array([t[1] for t in qw])

    slots = []        # dict(lvl, q, wq, k, nch, k_last, Wl)
    slot_boxes = []   # [box_id or -1 per core]

    def cover_of(grp):
        w_max = int(max(w_ext[g] for g in grp))
        h_max = int(max(h_ext[g] for g in grp))
        best = None
        for qq in QS:
            for wq in range(-(-w_max // qq), 129):
                c = _shape_cost(w_max, h_max, qq, wq)
                if c is not None and (best is None or c < best[0]):
                    best = (c, qq, wq)
        return best

    def emit_group(l, grp):
        best = cover_of(grp)
        if best is None:
            # no single feasible shape covers this mix; split by width
            srt = sorted(grp, key=lambda g: -w_ext[g])
            half = max(1, len(srt) // 2)
            emit_group(l, srt[:half])
            emit_group(l, srt[half:])
            return
        _, qj, wqj = best
        kj = 128 // wqj
        h_max = int(max(h_ext[g] for g in grp))
        nchj = -(-h_max // kj)
        klj = max(1, h_max - (nchj - 1) * kj)
        cores_boxes = [-1] * N_CORES
        for c, bid in enumerate(grp):
            cores_boxes[c] = int(bid)
        slots.append(dict(lvl=l, q=qj, wq=wqj, k=kj, nch=nchj,
                          k_last=klj, Wl=LVL_HW[l][1]))
        slot_boxes.append(cores_boxes)

    widths = wqsel * qsel
    for l in range(4):
        ids = np.nonzero(lvl == l)[0]
        order = ids[np.lexsort((-h_ext[ids], -widths[ids]))]
        n_ranks = -(-len(order) // N_CORES)
        for j in range(n_ranks):
            emit_group(l, list(order[j * N_CORES:(j + 1) * N_CORES]))
    return slots, slot_boxes


def _host_prep(f0, f1, f2, f3, boxes, bidx):
    geo = _geometry(boxes, bidx)
    slots, slot_boxes = _plan(geo)
    n_slots = len(slots)

    # channels-last bf16 table
    segs = []
    for f in (f0, f1, f2, f3):
        fa = np.asarray(f, np.float32)
        for bb in range(2):
            segs.append(np.transpose(fa[bb], (1, 2, 0)).reshape(-1, C))
    segs.append(np.zeros((PAD_PX, C), np.float32))
    table = np.ascontiguousarray(
        np.concatenate(segs, 0)).astype(nbf16).reshape(-1)

    # chunk bookkeeping: engine = slot parity; meta cols ordered
    # [all engine-0 chunks in emission order | all engine-1 chunks]
    eng_nch = [0, 0]
    wcols = 0
    for si, s in enumerate(slots):
        eng_nch[si % 2] += s["nch"]
        wcols += s["nch"] * s["q"] * NBIN
    tot_chunks = eng_nch[0] + eng_nch[1]

    metas = [np.zeros((1, tot_chunks), np.int32) for _ in range(N_CORES)]
    whosts = [np.zeros((128, wcols), np.float32) for _ in range(N_CORES)]

    ecur = [0, eng_nch[0]]
    colbase = 0
    for si, s in enumerate(slots):
        q, wq, k, nch, klast, Wl = (s["q"], s["wq"], s["k"], s["nch"],
                                    s["k_last"], s["Wl"])
        e = si % 2
        mbase = ecur[e]
        ecur[e] += nch
        rows_tot = (nch - 1) * k + klast
        for core in range(N_CORES):
            bid = slot_boxes[si][core]
            if bid < 0:
                continue
            l, bimg = geo["lvl"][bid], geo["b"][bid]
            base_px = (SEG_BASE[l, bimg] + geo["ymin"][bid] * Wl
                       + geo["xmin"][bid])
            for cch in range(nch):
                metas[core][0, mbase + cch] = (base_px + cch * k * Wl) * C
            h = int(geo["h_ext"][bid])
            w = int(geo["w_ext"][bid])
            Ap = np.zeros((OUT, rows_tot), np.float32)
            Ap[:, :h] = geo["A"][bid][:, :min(h, rows_tot)]
            Bp = np.zeros((OUT, wq * q), np.float32)
            Bp[:, :w] = geo["B"][bid]
            T = np.einsum('ar,bd->rdab', Ap, Bp).reshape(rows_tot, wq, q,
                                                         NBIN)
            wh = whosts[core]
            for cch in range(nch):
                rows_c = k if cch < nch - 1 else klast
                arr = T[cch * k:cch * k + rows_c]      # [rows_c, wq, q, 49]
                used = rows_c * wq
                cb = colbase + cch * q * NBIN
                wh[:used, cb:cb + q * NBIN] = arr.reshape(used, q * NBIN)
        colbase += nch * q * NBIN
    assert colbase == wcols

    whosts = [w.astype(nbf16) for w in whosts]
    key = tuple((s["lvl"], s["q"], s["wq"], s["nch"], s["k_last"])
                for s in slots)
    return (table, metas, whosts, slots, slot_boxes, tot_chunks, eng_nch,
            n_slots, key)


def _build_nc(slots, eng_nch):
    tot_chunks = eng_nch[0] + eng_nch[1]
    n_slots = len(slots)
    wcols = sum(s["nch"] * s["q"] * NBIN for s in slots)
    nc = bacc.Bacc("TRN2", target_bir_lowering=False, debug=False,
                   num_devices=N_CORES)
    table_d = nc.dram_tensor("table", [TABLE_PX * C], BF16,
                             kind="ExternalInput")
    meta_d = nc.dram_tensor("meta", [1, tot_chunks], mybir.dt.int32,
                            kind="ExternalInput")
    w_d = nc.dram_tensor("wts", [128, wcols], BF16, kind="ExternalInput")
    out_d = nc.dram_tensor("out", [NBIN, n_slots, C], BF16,
                           kind="ExternalOutput")

    with tile.TileContext(nc) as tc, ExitStack() as ctx:
        sbm = ctx.enter_context(tc.tile_pool(name="sbm", bufs=1))
        sbw = ctx.enter_context(tc.tile_pool(name="sbw", bufs=2))
        sbp = ctx.enter_context(tc.tile_pool(name="sbp", bufs=PATCH_BUFS))
        sbo = ctx.enter_context(tc.tile_pool(name="sbo", bufs=2))
        psp = ctx.enter_context(tc.psum_pool(name="psp", bufs=8))

        meta_t = sbm.tile([1, tot_chunks], mybir.dt.int32)
        nc.sync.dma_start(out=meta_t[:], in_=meta_d.ap())

        engs = [nc.sync, nc.scalar]
        # batched offset loads: per engine, a rolling window of ScalarValues
        ecur = [0, eng_nch[0]]
        eoff_vals = [{}, {}]

        def get_off(e, mi):
            vals = eoff_vals[e]
            if mi not in vals:
                b0 = mi - (mi - (0 if e == 0 else eng_nch[0])) % VLOAD_B
                n = min(VLOAD_B,
                        (eng_nch[0] if e == 0 else tot_chunks) - b0)
                regs = [engs[e].alloc_register(f"off{e}_{b0}_{i}")
                        for i in range(n)]
                engs[e].reg_load(regs, meta_t[0:1, b0:b0 + n])
                for i, r in enumerate(regs):
                    vals[b0 + i] = engs[e].snap(r, donate=True)
            return vals.pop(mi)

        # W streaming blocks at chunk granularity
        wblocks = []        # (colstart, colend)
        chunk_wcol = []     # per global chunk-col ranges are tracked inline
        wtiles = {}

        # precompute chunk -> (block, col) assignment
        col = 0
        chunk_cols = []     # per (si, cch): col
        for s in slots:
            for cch in range(s["nch"]):
                chunk_cols.append((col, s["q"] * NBIN))
                col += s["q"] * NBIN
        blocks = []
        b0 = 0
        for gci, (c0, cw) in enumerate(chunk_cols):
            if c0 + cw - b0 > WBLK_COLS:
                blocks.append((b0, c0))
                b0 = c0
        blocks.append((b0, wcols))
        chunk_blk = {}
        for gci, (c0, cw) in enumerate(chunk_cols):
            for bi, (lo, hi) in enumerate(blocks):
                if lo <= c0 < hi:
                    chunk_blk[gci] = bi
                    break

        def get_wblk(bi):
            if bi not in wtiles:
                lo, hi = blocks[bi]
                t = sbw.tile([128, hi - lo], BF16)
                nc.sync.dma_start(out=t[:], in_=w_d.ap()[:, lo:hi])
                wtiles[bi] = (t, lo)
            return wtiles[bi]

        stage = None
        gsz = 0
        gci = 0
        for si, s in enumerate(slots):
            q, wq, k, nch, klast, Wl = (s["q"], s["wq"], s["k"], s["nch"],
                                        s["k_last"], s["Wl"])
            e = si % 2
            eng = engs[e]
            qC = q * C
            patch = sbp.tile([128, PFREE], BF16)
            for cch in range(nch):
                rows_c = k if cch < nch - 1 else klast
                off = get_off(e, ecur[e])
                ecur[e] += 1
                src = bass.AP(tensor=table_d, offset=off,
                              ap=[[Wl * C, rows_c], [qC, wq], [1, qC]])
                eng.dma_start(out=patch[:rows_c * wq,
                                        cch * qC:(cch + 1) * qC], in_=src)
            ps = psp.tile([NBIN, C], mybir.dt.float32)
            for cch in range(nch):
                rows_c = k if cch < nch - 1 else klast
                kc = rows_c * wq
                bi = chunk_blk[gci]
                wt, wlo = get_wblk(bi)
                c0 = chunk_cols[gci][0] - wlo
                for sub in range(q):
                    nc.tensor.matmul(
                        out=ps[:],
                        lhsT=wt[:kc, c0 + sub * NBIN:c0 + (sub + 1) * NBIN],
                        rhs=patch[:kc, cch * qC + sub * C:
                                  cch * qC + (sub + 1) * C],
                        start=(cch == 0 and sub == 0),
                        stop=(cch == nch - 1 and sub == q - 1))
                gci += 1
            gi = si % OUT_GRP
            if gi == 0:
                gsz = min(OUT_GRP, n_slots - si)
                stage = sbo.tile([NBIN, gsz, C], BF16)
            nc.vector.tensor_scalar_mul(stage[:, gi, :], ps[:], 1.0)
            if gi == gsz - 1:
                g0 = si - gi
                nc.scalar.dma_start(
                    out=out_d.ap()[:, g0:g0 + gsz, :], in_=stage[:])
    nc.compile()
    return nc


LAST_RESULT = None


def kernel(f0, f1, f2, f3, boxes, box_batch_idx):
    global LAST_RESULT
    (table, metas, whosts, slots, slot_boxes, tot_chunks, eng_nch, n_slots,
     key) = _host_prep(f0, f1, f2, f3, boxes, box_batch_idx)
    if key not in _nc_cache:
        _nc_cache[key] = _build_nc(slots, eng_nch)
    nc = _nc_cache[key]
    in_maps = [{"table": table, "meta": metas[i], "wts": whosts[i]}
               for i in range(N_CORES)]
    res = bass_utils.run_bass_kernel_spmd(nc, in_maps,
                                          core_ids=list(range(N_CORES)))
    LAST_RESULT = res

    outfull = np.zeros((1024, NBIN, C), np.float32)
    for core in range(N_CORES):
        r = np.asarray(res.results[core]["out"]).astype(np.float32)
        r = r.reshape(NBIN, n_slots, C)
        for si in range(n_slots):
            bid = slot_boxes[si][core]
            if bid >= 0:
                outfull[bid] = r[:, si, :]
    return np.ascontiguousarray(
        outfull.transpose(0, 2, 1).reshape(1024, C, OUT, OUT))


# revision 24
# speedup vs baseline: 2.6612x; 1.6513x over previous
"""FPN ROIAlign pooler (nn_Pooler) on 8 trn2 cores — TensorE version.

Host builds a channels-last bf16 pixel table and per-box separable bilinear
weights collapsed into dense per-chunk weight matrices W[128slots, q*49].
Device, per box: dynamic-offset strided DMAs load the footprint as q-pixel
"slots" (q in {2,4,8} -> 1-4KB descriptors) into [128, nch, q*256] tiles;
q*nch matmuls W^T @ patch accumulate pooled[49, 256] in PSUM; DVE evacuates
(cast bf16) into staged groups DMAd out. Offsets are loaded 16-at-a-time into
registers (one TENSOR_LOAD each). Boxes sorted per level and dealt
round-robin so all 8 cores run one shared program; per-rank shape covers are
maxed over cores.
"""
import numpy as np
import ml_dtypes
from contextlib import ExitStack

from concourse import bacc, bass, mybir, tile, bass_utils

nbf16 = ml_dtypes.bfloat16
BF16 = mybir.dt.bfloat16
C = 256
OUT = 7
NBIN = OUT * OUT
N_CORES = 8
LVL_HW = [(200, 304), (100, 152), (50, 76), (25, 38)]
SCALES = (0.25, 0.125, 0.0625, 0.03125)
SEG_BASE = np.zeros((4, 2), np.int64)
_off = 0
for _l in range(4):
    for _b in range(2):
        SEG_BASE[_l, _b] = _off
        _off += LVL_HW[_l][0] * LVL_HW[_l][1]
TOTAL_PX = int(_off)
PAD_PX = 40960
TABLE_PX = TOTAL_PX + PAD_PX

# row widths (in q-px slots) whose k*wq fill is >= 120 of 128 partitions
WQ_SET = [1, 2, 3, 4, 5, 6, 8, 10, 12, 16, 20, 21, 24, 25, 32, 40, 42,
          60, 64, 128]

QS = (2, 4, 8)
VLOAD_B = 16           # offsets per batched register load
WBLK_COLS = 8192       # W streaming block width (bf16 cols)
OUT_GRP = 8            # slots per output staging group
PFREE = 4096           # constant patch-tile free dim (elems, bf16)
PATCH_BUFS = 6

_nc_cache = {}


def _geometry(boxes, bidx):
    """Per-box level routing + footprint + separable bilinear weights."""
    boxes32 = np.asarray(boxes, np.float32)
    b = np.asarray(bidx).astype(np.int64)
    N = boxes32.shape[0]

    x1, y1, x2, y2 = (boxes32[:, k] for k in range(4))
    area = (x2 - x1 + np.float32(1.0)) * (y2 - y1 + np.float32(1.0))
    s = np.sqrt(area)
    lv = np.floor(np.float32(4.0) + np.log2(s / np.float32(224.0)
                                            + np.float32(1e-6)))
    lvl = (np.clip(lv, 2.0, 5.0)).astype(np.int64) - 2

    scs = np.array(SCALES)[lvl]
    Wl = np.array([hw[1] for hw in LVL_HW])[lvl]
    Hl = np.array([hw[0] for hw in LVL_HW])[lvl]
    x1s = boxes32[:, 0].astype(np.float64) * scs
    y1s = boxes32[:, 1].astype(np.float64) * scs
    x2s = boxes32[:, 2].astype(np.float64) * scs
    y2s = boxes32[:, 3].astype(np.float64) * scs
    bin_w = np.maximum(x2s - x1s, 1.0) / OUT
    bin_h = np.maximum(y2s - y1s, 1.0) / OUT
    grid = (np.arange(OUT)[:, None]
            + np.array([0.25, 0.75])[None, :]).reshape(-1)
    xs = x1s[:, None] + bin_w[:, None] * grid[None, :]     # [N,14]
    ys = y1s[:, None] + bin_h[:, None] * grid[None, :]
    vx = (xs >= -1.0) & (xs <= Wl[:, None])
    vy = (ys >= -1.0) & (ys <= Hl[:, None])
    xc = np.clip(xs, 0.0, (Wl - 1)[:, None])
    yc = np.clip(ys, 0.0, (Hl - 1)[:, None])
    x0c = np.minimum(np.floor(xc).astype(np.int64), (Wl - 2)[:, None])
    y0c = np.minimum(np.floor(yc).astype(np.int64), (Hl - 2)[:, None])
    lx = xc - x0c
    ly = yc - y0c

    xmin = x0c.min(1)
    ymin = y0c.min(1)
    w_ext = x0c.max(1) + 1 - xmin + 1
    h_ext = y0c.max(1) + 1 - ymin + 1

    A_list, B_list = [], []
    for i in range(N):
        A = np.zeros((OUT, int(h_ext[i])), np.float64)
        Bm = np.zeros((OUT, int(w_ext[i])), np.float64)
        ry = y0c[i] - ymin[i]
        rx = x0c[i] - xmin[i]
        wy0 = 0.5 * vy[i] * (1.0 - ly[i])
        wy1 = 0.5 * vy[i] * ly[i]
        wx0 = 0.5 * vx[i] * (1.0 - lx[i])
        wx1 = 0.5 * vx[i] * lx[i]
        for sy in range(2 * OUT):
            by = sy // 2
            A[by, ry[sy]] += wy0[sy]
            A[by, ry[sy] + 1] += wy1[sy]
        for sx in range(2 * OUT):
            bx = sx // 2
            Bm[bx, rx[sx]] += wx0[sx]
            Bm[bx, rx[sx] + 1] += wx1[sx]
        A_list.append(A.astype(np.float32))
        B_list.append(Bm.astype(np.float32))

    return dict(lvl=lvl, b=b, xmin=xmin, ymin=ymin, w_ext=w_ext,
                h_ext=h_ext, A=A_list, B=B_list)


def _shape_cost(w, h, q, wq):
    """ns-scale cost of covering a (w x h) footprint with q-px slots,
    wq slots per row; every chunk reads full k rows (k*wq >= 120 keeps the
    SDMA engines balanced): SDMA bytes (reads + W) + PE + descriptors."""
    k = 128 // wq
    if k == 0:
        return None
    nch = -(-h // k)
    if nch * q * C > PFREE:
        return None
    read_b = nch * k * wq * q * 512
    w_b = nch * 128 * q * 98
    pe = nch * q * 305 / 2.4
    return (read_b + w_b) / 358.0 + pe + 8.0 * nch * k * wq


def _best_qwq(w, h):
    best = None
    for q in QS:
        wq0 = -(-w // q)
        if wq0 > 128:
            continue
        for wq in WQ_SET:
            if wq < wq0:
                continue
            c = _shape_cost(w, h, q, wq)
            if c is not None and (best is None or c < best[0]):
                best = (c, q, wq)
    return best[1], best[2]


def _plan(geo):
    """Sort + deal boxes, compute per-rank shape covers."""
    lvl = geo["lvl"]
    w_ext = geo["w_ext"]
    h_ext = geo["h_ext"]
    N = len(w_ext)
    qw = [_best_qwq(int(w_ext[i]), int(h_ext[i])) for i in range(N)]
    qsel = np.array([t[0] for t in qw])
    wqsel = np.array([t[1] for t in qw])

    slots = []        # dict(lvl, q, wq, k, nch, k_last, Wl)
    slot_boxes = []   # [box_id or -1 per core]

    def cover_of(grp):
        w_max = int(max(w_ext[g] for g in grp))
        h_max = int(max(h_ext[g] for g in grp))
        best = None
        for qq in QS:
            wq0 = -(-w_max // qq)
            for wq in WQ_SET:
                if wq < wq0:
                    continue
                c = _shape_cost(w_max, h_max, qq, wq)
                if c is not None and (best is None or c < best[0]):
                    best = (c, qq, wq)
        return best

    def emit_group(l, grp):
        best = cover_of(grp)
        if best is None:
            # no single feasible shape covers this mix; split by width
            srt = sorted(grp, key=lambda g: -w_ext[g])
            half = max(1, len(srt) // 2)
            emit_group(l, srt[:half])
            emit_group(l, srt[half:])
            return
        _, qj, wqj = best
        kj = 128 // wqj
        h_max = int(max(h_ext[g] for g in grp))
        nchj = -(-h_max // kj)
        klj = kj        # all chunks read full k rows: balanced partitions
        cores_boxes = [-1] * N_CORES
        for c, bid in enumerate(grp):
            cores_boxes[c] = int(bid)
        slots.append(dict(lvl=l, q=qj, wq=wqj, k=kj, nch=nchj,
                          k_last=klj, Wl=LVL_HW[l][1]))
        slot_boxes.append(cores_boxes)

    widths = wqsel * qsel
    for l in range(4):
        ids = np.nonzero(lvl == l)[0]
        order = ids[np.lexsort((-h_ext[ids], -widths[ids]))]
        n_ranks = -(-len(order) // N_CORES)
        for j in range(n_ranks):
            emit_group(l, list(order[j * N_CORES:(j + 1) * N_CORES]))
    return slots, slot_boxes


def _host_prep(f0, f1, f2, f3, boxes, bidx):
    geo = _geometry(boxes, bidx)
    slots, slot_boxes = _plan(geo)
    n_slots = len(slots)

    # channels-last bf16 table
    segs = []
    for f in (f0, f1, f2, f3):
        fa = np.asarray(f, np.float32)
        for bb in range(2):
            segs.append(np.transpose(fa[bb], (1, 2, 0)).reshape(-1, C))
    segs.append(np.zeros((PAD_PX, C), np.float32))
    table = np.ascontiguousarray(
        np.concatenate(segs, 0)).astype(nbf16).reshape(-1)

    # chunk bookkeeping: engine = slot parity; meta cols ordered
    # [all engine-0 chunks in emission order | all engine-1 chunks]
    eng_nch = [0, 0]
    wcols = 0
    for si, s in enumerate(slots):
        eng_nch[si % 2] += s["nch"]
        wcols += s["nch"] * s["q"] * NBIN
    tot_chunks = eng_nch[0] + eng_nch[1]

    metas = [np.zeros((1, tot_chunks), np.int32) for _ in range(N_CORES)]
    whosts = [np.zeros((128, wcols), np.float32) for _ in range(N_CORES)]

    ecur = [0, eng_nch[0]]
    colbase = 0
    for si, s in enumerate(slots):
        q, wq, k, nch, klast, Wl = (s["q"], s["wq"], s["k"], s["nch"],
                                    s["k_last"], s["Wl"])
        e = si % 2
        mbase = ecur[e]
        ecur[e] += nch
        rows_tot = (nch - 1) * k + klast
        for core in range(N_CORES):
            bid = slot_boxes[si][core]
            if bid < 0:
                continue
            l, bimg = geo["lvl"][bid], geo["b"][bid]
            base_px = (SEG_BASE[l, bimg] + geo["ymin"][bid] * Wl
                       + geo["xmin"][bid])
            for cch in range(nch):
                metas[core][0, mbase + cch] = (base_px + cch * k * Wl) * C
            h = int(geo["h_ext"][bid])
            w = int(geo["w_ext"][bid])
            Ap = np.zeros((OUT, rows_tot), np.float32)
            Ap[:, :h] = geo["A"][bid][:, :min(h, rows_tot)]
            Bp = np.zeros((OUT, wq * q), np.float32)
            Bp[:, :w] = geo["B"][bid]
            T = np.einsum('ar,bd->rdab', Ap, Bp).reshape(rows_tot, wq, q,
                                                         NBIN)
            wh = whosts[core]
            for cch in range(nch):
                rows_c = k if cch < nch - 1 else klast
                arr = T[cch * k:cch * k + rows_c]      # [rows_c, wq, q, 49]
                used = rows_c * wq
                cb = colbase + cch * q * NBIN
                wh[:used, cb:cb + q * NBIN] = arr.reshape(used, q * NBIN)
        colbase += nch * q * NBIN
    assert colbase == wcols

    whosts = [w.astype(nbf16) for w in whosts]
    key = tuple((s["lvl"], s["q"], s["wq"], s["nch"], s["k_last"])
                for s in slots)
    return (table, metas, whosts, slots, slot_boxes, tot_chunks, eng_nch,
            n_slots, key)


def _build_nc(slots, eng_nch):
    tot_chunks = eng_nch[0] + eng_nch[1]
    n_slots = len(slots)
    wcols = sum(s["nch"] * s["q"] * NBIN for s in slots)
    nc = bacc.Bacc("TRN2", target_bir_lowering=False, debug=False,
                   num_devices=N_CORES)
    table_d = nc.dram_tensor("table", [TABLE_PX * C], BF16,
                             kind="ExternalInput")
    meta_d = nc.dram_tensor("meta", [1, tot_chunks], mybir.dt.int32,
                            kind="ExternalInput")
    w_d = nc.dram_tensor("wts", [128, wcols], BF16, kind="ExternalInput")
    out_d = nc.dram_tensor("out", [NBIN, n_slots, C], BF16,
                           kind="ExternalOutput")

    with tile.TileContext(nc) as tc, ExitStack() as ctx:
        sbm = ctx.enter_context(tc.tile_pool(name="sbm", bufs=1))
        sbw = ctx.enter_context(tc.tile_pool(name="sbw", bufs=2))
        sbp = ctx.enter_context(tc.tile_pool(name="sbp", bufs=PATCH_BUFS))
        sbo = ctx.enter_context(tc.tile_pool(name="sbo", bufs=2))
        psp = ctx.enter_context(tc.psum_pool(name="psp", bufs=8))

        meta_t = sbm.tile([1, tot_chunks], mybir.dt.int32)
        nc.sync.dma_start(out=meta_t[:], in_=meta_d.ap())

        engs = [nc.sync, nc.scalar]
        # batched offset loads: per engine, a rolling window of ScalarValues
        ecur = [0, eng_nch[0]]
        eoff_vals = [{}, {}]

        def get_off(e, mi):
            vals = eoff_vals[e]
            if mi not in vals:
                b0 = mi - (mi - (0 if e == 0 else eng_nch[0])) % VLOAD_B
                n = min(VLOAD_B,
                        (eng_nch[0] if e == 0 else tot_chunks) - b0)
                regs = [engs[e].alloc_register(f"off{e}_{b0}_{i}")
                        for i in range(n)]
                engs[e].reg_load(regs, meta_t[0:1, b0:b0 + n])
                for i, r in enumerate(regs):
                    vals[b0 + i] = engs[e].snap(r, donate=True)
            return vals.pop(mi)

        # W streaming blocks at chunk granularity
        wblocks = []        # (colstart, colend)
        chunk_wcol = []     # per global chunk-col ranges are tracked inline
        wtiles = {}

        # precompute chunk -> (block, col) assignment
        col = 0
        chunk_cols = []     # per (si, cch): col
        for s in slots:
            for cch in range(s["nch"]):
                chunk_cols.append((col, s["q"] * NBIN))
                col += s["q"] * NBIN
        blocks = []
        b0 = 0
        for gci, (c0, cw) in enumerate(chunk_cols):
            if c0 + cw - b0 > WBLK_COLS:
                blocks.append((b0, c0))
                b0 = c0
        blocks.append((b0, wcols))
        chunk_blk = {}
        for gci, (c0, cw) in enumerate(chunk_cols):
            for bi, (lo, hi) in enumerate(blocks):
                if lo <= c0 < hi:
                    chunk_blk[gci] = bi
                    break

        def get_wblk(bi):
            if bi not in wtiles:
                lo, hi = blocks[bi]
                t = sbw.tile([128, hi - lo], BF16)
                nc.sync.dma_start(out=t[:], in_=w_d.ap()[:, lo:hi])
                wtiles[bi] = (t, lo)
            return wtiles[bi]

        stage = None
        gsz = 0
        gci = 0
        for si, s in enumerate(slots):
            q, wq, k, nch, klast, Wl = (s["q"], s["wq"], s["k"], s["nch"],
                                        s["k_last"], s["Wl"])
            e = si % 2
            eng = engs[e]
            qC = q * C
            patch = sbp.tile([128, PFREE], BF16)
            for cch in range(nch):
                rows_c = k if cch < nch - 1 else klast
                off = get_off(e, ecur[e])
                ecur[e] += 1
                src = bass.AP(tensor=table_d, offset=off,
                              ap=[[Wl * C, rows_c], [qC, wq], [1, qC]])
                eng.dma_start(out=patch[:rows_c * wq,
                                        cch * qC:(cch + 1) * qC], in_=src)
            ps = psp.tile([NBIN, C], mybir.dt.float32)
            for cch in range(nch):
                rows_c = k if cch < nch - 1 else klast
                kc = rows_c * wq
                bi = chunk_blk[gci]
                wt, wlo = get_wblk(bi)
                c0 = chunk_cols[gci][0] - wlo
                for sub in range(q):
                    nc.tensor.matmul(
                        out=ps[:],
                        lhsT=wt[:kc, c0 + sub * NBIN:c0 + (sub + 1) * NBIN],
                        rhs=patch[:kc, cch * qC + sub * C:
                                  cch * qC + (sub + 1) * C],
                        start=(cch == 0 and sub == 0),
                        stop=(cch == nch - 1 and sub == q - 1))
                gci += 1
            gi = si % OUT_GRP
            if gi == 0:
                gsz = min(OUT_GRP, n_slots - si)
                stage = sbo.tile([NBIN, gsz, C], BF16)
            nc.vector.tensor_scalar_mul(stage[:, gi, :], ps[:], 1.0)
            if gi == gsz - 1:
                g0 = si - gi
                nc.scalar.dma_start(
                    out=out_d.ap()[:, g0:g0 + gsz, :], in_=stage[:])
    nc.compile()
    return nc


LAST_RESULT = None


def kernel(f0, f1, f2, f3, boxes, box_batch_idx):
    global LAST_RESULT
    (table, metas, whosts, slots, slot_boxes, tot_chunks, eng_nch, n_slots,
     key) = _host_prep(f0, f1, f2, f3, boxes, box_batch_idx)
    if key not in _nc_cache:
        _nc_cache[key] = _build_nc(slots, eng_nch)
    nc = _nc_cache[key]
    in_maps = [{"table": table, "meta": metas[i], "wts": whosts[i]}
               for i in range(N_CORES)]
    res = bass_utils.run_bass_kernel_spmd(nc, in_maps,
                                          core_ids=list(range(N_CORES)))
    LAST_RESULT = res

    outfull = np.zeros((1024, NBIN, C), np.float32)
    for core in range(N_CORES):
        r = np.asarray(res.results[core]["out"]).astype(np.float32)
        r = r.reshape(NBIN, n_slots, C)
        for si in range(n_slots):
            bid = slot_boxes[si][core]
            if bid >= 0:
                outfull[bid] = r[:, si, :]
    return np.ascontiguousarray(
        outfull.transpose(0, 2, 1).reshape(1024, C, OUT, OUT))
